# revision 16
# baseline (speedup 1.0000x reference)
"""MDCT kernel for Trainium2 (8 NeuronCores, batch-parallel), folded DCT-IV form.

Math: frame f (hop N=1024, len 2N, center-padded) folds to an N-vector u and
out[f] = DCT-IV(u).  With x2 = x.reshape(1024, 1024) and y1 = w[:N]*x2[r],
y2 = w[N:]*x2[r] (per-row windowing):
    u[f, m]      = -y2[f, 511-m] - y2[f, 512+m]      (m < 512,  row f)
    u[f, 512+p]  =  y1[f-1, p]   - y1[f-1, 1023-p]   (p < 512,  row f-1)
so each x2 row r yields uLo[r] (frame r) and uHi[r] (frame r+1), and
    out[f, k] = sum_m u[f, m] * D4[m, k],   D4 = sqrt(2/N) DCT-IV matrix.

Only the LEFT half of D4 is shipped (1.05 MB instead of 2.1 MB), using
    D4[m, 512+k] = s_m*sqrt(2)*D4[m, k] - D4[m, 511-k],
    s_m = +1 for m%4 in {0,3} else -1,
which for the output means
    out[f, 512+k] = pa'[f, k] - pa[f, 511-k]
where pa = u^T Dl (the left-half chain) and pa' is the SAME chain with
sign-scaled weights u' = sqrt(2)*s*u.  u' costs nothing on the PE: the
scale is applied by the PSUM->SBUF staging copies (per-partition scale AP
on ACT, tensor_scalar on DVE), and the final combine (pa' minus
column-reversed pa) replaces the plain pb copy at identical DVE cost.
So PE work is identical to the full-D kernel while the DCT-gating fill
drops by 1.05 MB (~2.7 us at the ~400 GB/s shared-DMA rate).

Schedule notes (v7; NTFF profiles: baseline=53.8us, batched-DMA=66.8us,
gpsimd-derive=124us, v4=57.6us, PE-derive=69.7/61.8us):
- the NEFF preamble ends ~7.2 us; nothing (not even DMA) starts earlier.
- concurrent DMA streams share ~400 GB/s; a transfer completes when the
  cumulative bytes ahead of it have streamed, so the DCT gate is the
  byte count of x0+w+x1+Dl (~2.05 MB), cleared ~12.8 us.
- warmup transposes keep the PE continuously busy from the preamble to
  the first fold transposes: any >3.4 us PE idle re-throttles the HAM
  clock to 1.2 GHz and the ramp back takes ~3.5 us of sustained work.
- proven baseline software pipeline: fold(r+2) is emitted before
  dct_tile(r) so fold transposes interleave into chain-link stalls.
- engine ownership: DVE = folds + uLo staging + the pa'-pa combine;
  ACT = uHi/uHi'/uLo' staging + pa copy + a-half stores (engine-local);
  Sync = fills + b-half stores.  GPSIMD is unusable (no PSUM access,
  ~7.6us per [128,512] tensor_scalar).
- frame 1024 (uHi of row 1023 only) runs as a 1-partition chain at the
  very end; its copies/stores are tiny so the drain tail is short.
  Engine APs cannot start at partition 1, so it cannot become a
  shifted-psum combine.
"""

import numpy as np
import ml_dtypes

import concourse.bass as bass
import concourse.bacc as bacc
import concourse.mybir as mybir
import concourse.tile as tile
from concourse import masks
from concourse.bass_utils import run_bass_kernel_spmd

B = 8
T = 1 << 20
R = 1024          # rows of x2 per channel (T // hop)
CN = 1024         # row width (hop) = N
NF = 1025         # output frames
NK = 1024         # output bins
F32 = mybir.dt.float32
BF16 = mybir.dt.bfloat16

_NC_CACHE = None
_CONST_CACHE = None


def build_nc() -> bass.Bass:
    nc = bacc.Bacc("TRN2", target_bir_lowering=False, debug=False)
    x = nc.dram_tensor("x", [R, CN], BF16, kind="ExternalInput").ap()
    wcr = nc.dram_tensor("wcr", [128, 2 * CN], BF16, kind="ExternalInput").ap()
    d4l = nc.dram_tensor("d4l", [8, 128, 512], BF16, kind="ExternalInput").ap()
    svr = nc.dram_tensor("svr", [128, 1], F32, kind="ExternalInput").ap()
    out = nc.dram_tensor("out", [NF, NK], BF16, kind="ExternalOutput").ap()

    xv = x.rearrange("(a p) c -> p a c", p=128)
    dv = d4l.rearrange("a p c -> p a c")

    with tile.TileContext(nc) as tc:
        with (
            tc.tile_pool(name="persist", bufs=1) as persist,
            tc.tile_pool(name="xin", bufs=1) as xin,
            tc.tile_pool(name="ypool", bufs=6) as ypool,
            tc.tile_pool(name="upool", bufs=4) as upool,
            tc.tile_pool(name="outp", bufs=4) as outp,
            tc.tile_pool(name="wps", bufs=1, space="PSUM") as wps,
            tc.tile_pool(name="tps", bufs=2, space="PSUM") as tps,
            tc.tile_pool(name="mmps", bufs=4, space="PSUM") as mmps,
        ):
            wc = persist.tile([128, 2 * CN], BF16)
            w1 = wc[:, 0:CN]
            w2n = wc[:, CN:2 * CN]
            sv = persist.tile([128, 1], F32)

            ident = persist.tile([128, 128], BF16)
            masks.make_identity(nc, ident[:])

            dt = persist.tile([128, 8, 512], BF16)
            ulot = persist.tile([128, 4, R], BF16)
            uhit = persist.tile([128, 4, NF], BF16)
            ulotp = persist.tile([128, 4, R], BF16)
            uhitp = persist.tile([128, 4, NF], BF16)
            nc.vector.memset(uhit[:, :, 0:1], 0.0)
            nc.vector.memset(uhitp[:, :, 0:1], 0.0)

            xts = [xin.tile([128, CN], BF16, name=f"xt{i}") for i in range(8)]

            # PE warmup: keep the PE continuously busy from the preamble
            # barrier until fold(0)'s transposes, so the HAM clock gate
            # ramps to 2.4 GHz before the DCT stream starts.
            warm = wps.tile([128, 128], BF16, tag="warm")
            for _ in range(55):
                nc.tensor.transpose(warm[:], ident[:], ident[:])

            # Fill DMAs: DCT-gating bytes first, few big instructions so
            # the Sync issue rate (~0.65us each) never starves the DMA
            # engines.  The [128,1] sv const (128 four-byte packets) goes
            # on the idle GpSimd queue so it cannot block the stream.
            nc.gpsimd.dma_start(sv[:], svr)
            nc.sync.dma_start(xts[0][:], xv[:, 0, :])
            nc.sync.dma_start(wc[:], wcr)
            nc.sync.dma_start(xts[1][:], xv[:, 1, :])
            nc.sync.dma_start(dt[:], dv[:])
            for r in range(2, 8):
                nc.sync.dma_start(xts[r][:], xv[:, r, :])

            def fold(r: int):
                xt = xts[r][:]
                r0 = r * 128
                pt = tps.tile([128, CN], BF16, tag="pt")
                y1 = ypool.tile([128, CN], BF16, tag="y1")
                un = upool.tile([128, CN], BF16)
                nc.vector.tensor_tensor(y1[:], xt, w1[:], mybir.AluOpType.mult)
                # uHi[p] = y1[p] - y1[1023-p]
                nc.vector.tensor_tensor(
                    un[:, 512:1024], y1[:, 0:512], y1[:, 1023:511:-1],
                    mybir.AluOpType.subtract,
                )
                for ci in range(4):
                    nc.tensor.transpose(
                        pt[:, ci * 128:(ci + 1) * 128],
                        un[:, 512 + ci * 128:512 + (ci + 1) * 128], ident[:],
                    )
                nc.scalar.copy(uhit[:, 0:4, 1 + r0:1 + r0 + 128], pt[:, 0:512])
                nc.scalar.mul(uhitp[:, 0:4, 1 + r0:1 + r0 + 128], pt[:, 0:512],
                              sv[:, 0:1])
                y2n = ypool.tile([128, CN], BF16, tag="y2n")
                nc.vector.tensor_tensor(y2n[:], xt, w2n[:], mybir.AluOpType.mult)
                # uLo[m] = y2n[511-m] + y2n[512+m]   (y2n = -w2*x)
                nc.vector.tensor_tensor(
                    un[:, 0:512], y2n[:, 511::-1], y2n[:, 512:1024],
                    mybir.AluOpType.add,
                )
                for ci in range(4):
                    nc.tensor.transpose(
                        pt[:, 512 + ci * 128:512 + (ci + 1) * 128],
                        un[:, ci * 128:(ci + 1) * 128], ident[:],
                    )
                nc.vector.tensor_copy(ulot[:, 0:4, r0:r0 + 128], pt[:, 512:1024])
                nc.scalar.mul(ulotp[:, 0:4, r0:r0 + 128], pt[:, 512:1024],
                              sv[:, 0:1])

            def wslice(ci, f0, primed):
                lo, hi = (ulotp, uhitp) if primed else (ulot, uhit)
                if ci < 4:
                    return lo[:, ci, f0:f0 + 128]
                return hi[:, ci - 4, f0:f0 + 128]

            def dct_tile(j: int):
                f0 = j * 128
                ot = outp.tile([128, NK], BF16)
                pa = mmps.tile([128, 512], F32, tag="mm")
                for ci in range(8):
                    nc.tensor.matmul(
                        pa[:], wslice(ci, f0, False), dt[:, ci, :],
                        start=(ci == 0), stop=(ci == 7),
                    )
                nc.scalar.copy(ot[:, 0:512], pa[:])
                nc.gpsimd.dma_start(out[f0:f0 + 128, 0:512], ot[:, 0:512])
                pp = mmps.tile([128, 512], F32, tag="mm")
                for ci in range(8):
                    nc.tensor.matmul(
                        pp[:], wslice(ci, f0, True), dt[:, ci, :],
                        start=(ci == 0), stop=(ci == 7),
                    )
                # out[:, 512+k] = pa'[k] - pa[511-k]; the reversed pa
                # operand reads the SBUF copy (ot a-half) because engines
                # cannot read two PSUM inputs in one instruction.
                nc.vector.tensor_tensor(
                    ot[:, 512:1024], pp[:], ot[:, 511::-1],
                    mybir.AluOpType.subtract,
                )
                nc.sync.dma_start(out[f0:f0 + 128, 512:1024], ot[:, 512:1024])

            def last_frame():
                # f=1024: only the uHi half (row 1023) contributes.
                pa = mmps.tile([1, 512], F32, tag="mm")
                pp = mmps.tile([1, 512], F32, tag="mm")
                for ci in range(4):
                    nc.tensor.matmul(
                        pa[:], uhit[:, ci, 1024:1025], dt[:, 4 + ci, :],
                        start=(ci == 0), stop=(ci == 3),
                    )
                    nc.tensor.matmul(
                        pp[:], uhitp[:, ci, 1024:1025], dt[:, 4 + ci, :],
                        start=(ci == 0), stop=(ci == 3),
                    )
                ot = outp.tile([1, NK], BF16, tag="ot_last")
                nc.scalar.copy(ot[:, 0:512], pa[:])
                nc.gpsimd.dma_start(out[1024:1025, 0:512], ot[:, 0:512])
                nc.vector.tensor_tensor(
                    ot[:, 512:1024], pp[:], ot[:, 511::-1],
                    mybir.AluOpType.subtract,
                )
                nc.sync.dma_start(out[1024:1025, 512:1024], ot[:, 512:1024])

            fold(0)
            fold(1)
            for r in range(8):
                if r + 2 < 8:
                    fold(r + 2)
                dct_tile(r)
            last_frame()

    return nc


def make_consts(window: np.ndarray):
    w = window.astype(np.float64)
    wcr = np.broadcast_to(
        np.concatenate([w[:CN], -w[CN:]]).astype(ml_dtypes.bfloat16),
        (128, 2 * CN)).copy()
    m = np.arange(NK, dtype=np.float64)[:, None]
    k = np.arange(NK, dtype=np.float64)[None, :]
    d = (np.sqrt(2.0 / NK) * np.cos(np.pi / NK * (m + 0.5) * (k + 0.5)))
    d4l = np.ascontiguousarray(
        d.astype(ml_dtypes.bfloat16).reshape(8, 128, NK)[:, :, :512])
    p = np.arange(128)
    svr = np.where(np.isin(p % 4, [0, 3]), np.sqrt(2.0), -np.sqrt(2.0))
    svr = svr.reshape(128, 1).astype(np.float32)
    return wcr, d4l, svr


def _get_nc() -> bass.Bass:
    global _NC_CACHE
    if _NC_CACHE is None:
        _NC_CACHE = build_nc()
        _NC_CACHE.compile()
    return _NC_CACHE


def run_spmd(x: np.ndarray, window: np.ndarray, **kwargs):
    """Shard, run on 8 cores, return (stacked output, BassKernelResults)."""
    global _CONST_CACHE
    if _CONST_CACHE is None or _CONST_CACHE[0] != window.tobytes():
        _CONST_CACHE = (window.tobytes(), make_consts(window))
    wcr, d4l, svr = _CONST_CACHE[1]
    in_maps = [
        {"x": np.ascontiguousarray(
            x[b].reshape(R, CN).astype(ml_dtypes.bfloat16)),
         "wcr": wcr, "d4l": d4l, "svr": svr}
        for b in range(B)
    ]
    res = run_bass_kernel_spmd(nc=_get_nc(), in_maps=in_maps,
                               core_ids=list(range(B)), **kwargs)
    out = np.stack([res.results[b]["out"].astype(np.float32) for b in range(B)],
                   axis=0)
    return out, res


def kernel(x: np.ndarray, window: np.ndarray) -> np.ndarray:
    out, _ = run_spmd(np.asarray(x), np.asarray(window))
    return out
